# revision 46
# baseline (speedup 1.0000x reference)
"""DirectedLowRankEdgeScorer TRN2 Bass kernel (8 NeuronCores, SPMD), v2.

logits[b,l,e] = sum_r a[b,I[e],r] * gamma[l,r] * b[b,J[e],r]
  a = relu(H@W1s+b1s)@W2s+b2s,  b = relu(H@W1d+b1d)@W2d+b2d,  H = X[:,-1]

v2 design (vs. baseline): the kernel is SWDGE-descriptor-bound
(~2.1ns/desc across 4 queues), so everything targets descriptor count
and per-engine work:
  - fp16 node records: rec[n] = [a0 a1 b0 b1] (64 fp16 = 128B); the DRAM
    table is viewed as 256B rows holding TWO nodes (2r, 2r+1), indexed by
    r = node//2 (int16-safe without table splitting; halves kept only for
    AllGather overlap).
  - a-side PAIRING: two edges whose I fall in the same 2-node row share
    ONE 256B gather descriptor (~halves a-descs). Pairs are bucketed by
    (I%2,I%2,J%2,J%2) offset-class so the DVE product reads each class
    with one uniform strided AP. Leftover odd edges go to dedicated
    "singles" tiles (1 desc/edge both sides).
  - b-side: per-edge 256B descs in the pair-slot order.
  - DVE stream-transpose (32x32 blocks) replaces all PE transposes and
    the PSUM round-trip; single-pass fp16 gamma matmul; fp16 output
    upcast on host. fp16 MLP; fp16 AllGather (6.4MB total).
"""

import sys
import types

import numpy as np
import ml_dtypes

import bass_rust
import concourse.bass as bass
import concourse.bacc as bacc
import concourse.mybir as mybir
from concourse.bass import AP
from concourse.bass_utils import run_bass_kernel_spmd
from concourse.tile import TileContext
from concourse.vector_clock import ScopedClock
from concourse.tile import add_dep_helper

F16 = ml_dtypes.float16 if hasattr(ml_dtypes, "float16") else np.float16

B, T, N, C = 2, 8, 50000, 64
HID, R, L, E = 128, 16, 12, 1600000
NCORES = 8
NP = 6272                     # nodes per core shard (49*128)
NPAD = NP * NCORES            # 50176 padded node count
PRC = NP // 2                 # 3136 pair-rows per core
H1R, H2R = 1600, 1536         # pair-rows per core in phase1/phase2 tables
TP = 1024                     # pairs per paired tile (=2048 edges)
TS = 1024                     # edges per singles tile
SINGLE_PACKET = False         # SWDGE packet formation mode

# paired class id = 4*dai + 2*dj0 + dj1, dai in {0:(0,0),1:(0,1),2:(1,1)}
# singles class id = 2*di + dj
# device AP params per paired class: (a_off, a_dstride, b_off, b_dstride)
_DA = {0: (0, 0), 1: (0, 64), 2: (64, 0)}


def _pcls_ap(cls):
    dai, dj0, dj1 = cls // 4, (cls % 4) // 2, cls % 2
    a_off, a_ds = _DA[dai]
    b_off = 32 + 64 * dj0
    b_ds = 128 + 64 * (dj1 - dj0)
    return a_off, a_ds, b_off, b_ds


def _scls_ap(cls):
    di, dj = cls // 2, cls % 2
    return 64 * di, 32 + 64 * dj


# ---------------------------------------------------------------- patches
def _patched_drain_and_barrier(self, tick_clock, wait_clock):
    nc = self.nc
    probe = nc.sync.drain()
    wait_clock.add_sem_waits(probe.ins, ScopedClock({None: tick_clock.global_clock}))
    si = probe.ins.sync_info
    waits = list(si.on_wait) if si is not None else []
    if len(waits) > 2:
        si.on_wait.clear()
        si.on_wait.extend(waits[:2])
        for k in range(2, len(waits), 2):
            ni = nc.sync.drain().ins
            ni.sync_info = bass_rust.SyncInfo(on_wait=waits[k:k + 2], on_update=[])
    nc.all_engine_barrier()
    assert self.sems is not None
    popped = nc._tile_sem_poison_stack.pop()
    assert popped is self._sem_poison
    nc.clear_and_free_semaphores(list(self.sems.allocated().values()))
    nc.all_engine_barrier()


TileContext._drain_and_barrier = _patched_drain_and_barrier

if "antenv.axon_hooks" not in sys.modules:
    _mod = types.ModuleType("antenv.axon_hooks")
    _state = {"hook": None}
    _mod.set_axon_ntff_profile_hook = lambda h: _state.__setitem__("hook", h)
    _mod.get_axon_ntff_profile_hook = lambda: _state["hook"]
    sys.modules["antenv.axon_hooks"] = _mod
    try:
        import antenv

        antenv.axon_hooks = _mod
    except Exception:
        pass
    try:
        from trn_agent_boot.trn_boot import _ntff_profile_via_ctypes

        _hook = _ntff_profile_via_ctypes("/opt/axon/libaxon_pjrt.so")
        if _hook is not None:
            _mod.set_axon_ntff_profile_hook(_hook)
    except Exception:
        pass


# ---------------------------------------------------------------- device
_PROGRAM_CACHE = {}


def _view(base, extra_off_elems, dims):
    """Strided view of a tile AP: keep partition dim, replace free dims.

    dims: list of (stride_elems, size)."""
    return AP(base.tensor, base.offset + extra_off_elems, [tuple(base.ap[0])] + [
        (int(s), int(n)) for (s, n) in dims
    ])


def build_program(plan):
    """plan: tuple over tiles of (group, kind, runs)
    kind 0 = paired (2048 edges, 512 out cols), 1 = singles (1024 e, 256).
    runs: ((j0, j1, p0, p1, cls), ...) over a-slot/edge blocks; partition
    range (p0, p1) is (0, 128) whenever j1 > j0 + 1."""
    f32, f16, i16 = mybir.dt.float32, mybir.dt.float16, mybir.dt.int16
    nT = len(plan)
    totcols = sum(512 if t[1] == 0 else 256 for t in plan)

    nc = bacc.Bacc("TRN2", target_bir_lowering=False, num_swdge_queues=4)

    HT = nc.declare_dram_parameter("HT", [C, B, NP], f16, isOutput=False)
    W1 = nc.declare_dram_parameter("W1", [C, 2, HID], f16, isOutput=False)
    B1 = nc.declare_dram_parameter("B1", [HID, 2, 1], f32, isOutput=False)
    W2 = nc.declare_dram_parameter("W2", [HID, 2, R], f16, isOutput=False)
    B2 = nc.declare_dram_parameter("B2", [128, 2, R], f32, isOutput=False)
    GBD = nc.declare_dram_parameter("GBD", [128, 96], f16, isOutput=False)
    IDXA = nc.declare_dram_parameter("IDXA", [128, nT, 64], i16, isOutput=False)
    IDXB = nc.declare_dram_parameter("IDXB", [128, nT, 128], i16, isOutput=False)
    OUT = nc.declare_dram_parameter("OUT", [96, totcols], f16, isOutput=True)

    rec_shard = nc.dram_tensor("rec_shard", [NP, 64], f16)
    rec_h1 = nc.dram_tensor("rec_h1", [NCORES * H1R, 128], f16, addr_space="Shared")
    rec_h2 = nc.dram_tensor("rec_h2", [NCORES * H2R, 128], f16, addr_space="Shared")

    with TileContext(nc) as tc:
        with (
            tc.tile_pool(name="const", bufs=1) as constp,
            tc.tile_pool(name="h1p", bufs=1) as h1p,
            tc.tile_pool(name="recp", bufs=3) as recp,
            tc.tile_pool(name="gp", bufs=6) as gp,
            tc.tile_pool(name="prodp", bufs=4) as prodp,
            tc.tile_pool(name="ctp", bufs=4) as ctp,
            tc.tile_pool(name="outp", bufs=3) as outp,
            tc.tile_pool(name="psX", bufs=4, space="PSUM") as psX,
            tc.tile_pool(name="ps2", bufs=2, space="PSUM") as ps2,
            tc.tile_pool(name="psL", bufs=2, space="PSUM") as psL,
        ):
            w1_s = constp.tile([C, 2, HID], f16)
            nc.sync.dma_start(w1_s[:], W1[:])
            b1_s = constp.tile([HID, 2, 1], f32)
            nc.sync.dma_start(b1_s[:], B1[:])
            w2_s = constp.tile([HID, 2, R], f16)
            nc.sync.dma_start(w2_s[:], W2[:])
            b2_s = constp.tile([128, 2, R], f32)
            nc.sync.dma_start(b2_s[:], B2[:])
            ht_s = constp.tile([C, B, NP], f16)
            nc.sync.dma_start(ht_s[:], HT[:])
            gbd_s = constp.tile([128, 96], f16)
            nc.sync.dma_start(gbd_s[:], GBD[:])
            idxa_all = constp.tile([128, nT, 64], i16)
            nc.sync.dma_start(idxa_all[:], IDXA[:])
            idxb_all = constp.tile([128, nT, 128], i16)
            nc.sync.dma_start(idxb_all[:], IDXB[:])

            # ---- MLP passes; each ends with its half AllGather (fp16)
            cc_insts = []
            for (p0r, pszr) in ((0, H1R), (H1R, H2R)):
                n0, nn = 2 * p0r, 2 * pszr      # node offsets within shard
                h1t = {}
                for t in range(2):
                    for b in range(B):
                        h1x = h1p.tile([HID, 2 * max(H1R, H2R)], f16, tag=f"h1_{t}_{b}")
                        h1t[(t, b)] = h1x
                for c0 in range(0, nn, 512):
                    csz = min(512, nn - c0)
                    for t in range(2):
                        for b in range(B):
                            p1 = psX.tile([HID, 512], f32, tag="px")
                            nc.tensor.matmul(
                                p1[:, :csz],
                                w1_s[:, t, :],
                                ht_s[:, b, n0 + c0:n0 + c0 + csz],
                            )
                            nc.scalar.activation(
                                h1t[(t, b)][:, c0:c0 + csz], p1[:, :csz],
                                mybir.ActivationFunctionType.Relu,
                                bias=b1_s[:, t, :], scale=1.0,
                            )
                rec_dmas = []
                for s in range(nn // 128):
                    rec = recp.tile([128, 64], f16, tag="rec")
                    for t in range(2):
                        for b in range(B):
                            p2 = ps2.tile([128, R], f32, tag="p2")
                            nc.tensor.matmul(
                                p2[:],
                                h1t[(t, b)][:, s * 128:(s + 1) * 128],
                                w2_s[:, t, :],
                            )
                            co = 32 * t + 16 * b
                            with nc.allow_low_precision(reason="fp16 records"):
                                nc.vector.tensor_add(
                                    rec[:, co:co + 16], p2[:], b2_s[:, t, :]
                                )
                    m0 = n0 + s * 128
                    di = nc.sync.dma_start(rec_shard[m0:m0 + 128, :], rec[:])
                    rec_dmas.append(di)
                dst = rec_h1 if p0r == 0 else rec_h2
                cc = nc.gpsimd.collective_compute(
                    "AllGather",
                    mybir.AluOpType.bypass,
                    replica_groups=[list(range(NCORES))],
                    ins=[rec_shard[n0:n0 + nn, :]],
                    outs=[dst[:]],
                )
                for di in rec_dmas:
                    add_dep_helper(cc.ins, di.ins, True, "cc waits rec dmas")
                if cc_insts:
                    add_dep_helper(cc.ins, cc_insts[-1].ins, True, "cc order")
                cc_insts.append(cc)

            # ---- gather + score
            qctr = 0
            col0 = 0
            for Ti, (g, kind, runs) in enumerate(plan):
                recA = rec_h1 if g < 2 else rec_h2
                recB = rec_h1 if g % 2 == 0 else rec_h2
                ccA = cc_insts[0] if g < 2 else cc_insts[1]
                ccB = cc_insts[0] if g % 2 == 0 else cc_insts[1]

                gA = gp.tile([128, 8, 128], f16, tag="gA")
                ga_i = nc.gpsimd.dma_gather(
                    gA[:], recA[:], idxa_all[:, Ti, :],
                    num_idxs=1024, num_idxs_reg=1024, elem_size=128,
                    single_packet=SINGLE_PACKET, queue_num=qctr % 4,
                )
                qctr += 1
                add_dep_helper(ga_i.ins, ccA.ins, True, "gather waits cc")
                gB = gp.tile([128, 16, 128], f16, tag="gB")
                nkb = 2048 if kind == 0 else 1024
                gb_i = nc.gpsimd.dma_gather(
                    gB[:, :nkb // 128, :], recB[:],
                    idxb_all[:, Ti, :nkb // 16],
                    num_idxs=nkb, num_idxs_reg=nkb, elem_size=128,
                    single_packet=SINGLE_PACKET, queue_num=qctr % 4,
                )
                qctr += 1
                add_dep_helper(gb_i.ins, ccB.ins, True, "gather waits cc")

                ncols = 512 if kind == 0 else 256
                prod = prodp.tile([128, ncols], f16, tag=f"prod{kind}")
                if kind == 0:
                    for (j0, j1, p0, p1, cls) in runs:
                        a_off, a_ds, b_off, b_ds = _pcls_ap(cls)
                        nj = j1 - j0
                        in0 = _view(gA[:] if p1 - p0 == 128 else gA[p0:p1],
                                    j0 * 128 + a_off,
                                    [(128, nj), (a_ds, 2), (1, 32)])
                        in1 = _view(gB[:] if p1 - p0 == 128 else gB[p0:p1],
                                    j0 * 256 + b_off,
                                    [(256, nj), (b_ds, 2), (1, 32)])
                        out = _view(prod[:] if p1 - p0 == 128 else prod[p0:p1],
                                    j0 * 64,
                                    [(64, nj), (32, 2), (1, 32)])
                        with nc.allow_low_precision(reason="fp16 prod"):
                            nc.vector.tensor_mul(out, in0, in1)
                else:
                    for (j0, j1, p0, p1, cls) in runs:
                        a_off, b_off = _scls_ap(cls)
                        nj = j1 - j0
                        in0 = _view(gA[:] if p1 - p0 == 128 else gA[p0:p1],
                                    j0 * 128 + a_off,
                                    [(128, nj), (1, 32)])
                        in1 = _view(gB[:] if p1 - p0 == 128 else gB[p0:p1],
                                    j0 * 128 + b_off,
                                    [(128, nj), (1, 32)])
                        out = _view(prod[:] if p1 - p0 == 128 else prod[p0:p1],
                                    j0 * 32, [(32, nj), (1, 32)])
                        with nc.allow_low_precision(reason="fp16 prod"):
                            nc.vector.tensor_mul(out, in0, in1)

                pT = ctp.tile([128, ncols], f16, tag=f"pT{kind}")
                with nc.allow_low_precision(reason="fp16 transpose"):
                    nc.vector.transpose(pT[:], prod[:])

                pL = psL.tile([96, 512], f32, tag="pL")
                nc.tensor.matmul(pL[:, :ncols], gbd_s[:], pT[:])

                outS = outp.tile([96, ncols], f16, tag=f"outS{kind}")
                with nc.allow_low_precision(reason="fp16 out"):
                    nc.scalar.copy(outS[:], pL[:, :ncols])
                nc.sync.dma_start(OUT[:, col0:col0 + ncols], outS[:])
                col0 += ncols

    nc.finalize()
    return nc


# ---------------------------------------------------------------- host
def _wrap_idx(flat_idx):
    """[K] int16 -> [128, K//16] wrapped-16, replicated x8."""
    w = flat_idx.reshape(len(flat_idx) // 16, 16).T
    return np.tile(w, (8, 1))


def _seg_runs(bounds_cls):
    """bounds_cls: (s0, s1, cls) position segments within a tile ->
    ((j0, j1, p0, p1, cls), ...); partial-width runs split into
    size-aligned partition chunks (p % s == 0, s in {32, 64})."""
    out = []
    for (s0, s1, cl) in bounds_cls:
        a = s0
        while a < s1:
            j = a // 128
            b = min(s1, (j + 1) * 128)
            p0, p1 = a - 128 * j, b - 128 * j
            if (out and p0 == 0 and p1 == 128 and out[-1][2] == 0
                    and out[-1][3] == 128 and out[-1][1] == j
                    and out[-1][4] == cl):
                out[-1] = (out[-1][0], j + 1, 0, 128, cl)
            elif p0 == 0 and p1 == 128:
                out.append((j, j + 1, 0, 128, cl))
            else:
                p = p0
                while p < p1:
                    s = 64 if (p % 64 == 0 and p + 64 <= p1) else 32
                    out.append((j, j + 1, p, p + s, cl))
                    p += s
            a = b
    return tuple(out)


def kernel(X, edge_index, W1s, b1s, W2s, b2s, W1d, b1d, W2d, b2d, gamma):
    X = np.asarray(X)
    edge_index = np.asarray(edge_index)
    H = np.ascontiguousarray(X[:, -1]).astype(np.float32)          # (B, N, C)
    Hp = np.zeros((B, NPAD, C), np.float32)
    Hp[:, :N] = H

    I = edge_index[0].astype(np.int64)
    J = edge_index[1].astype(np.int64)

    # per-edge table coords
    def _rowhalf(nodes):
        rg = nodes // 2
        c = rg // PRC
        i = rg % PRC
        h = (i >= H1R).astype(np.int64)
        row = np.where(h == 0, H1R * c + i, H2R * c + (i - H1R))
        return row, h

    rowA, hI = _rowhalf(I)
    rowB, hJ = _rowhalf(J)
    dI = (I % 2).astype(np.int64)
    dJ = (J % 2).astype(np.int64)
    g_all = 2 * hI + hJ

    # ---- global pairing: sort by (g, rowA, dI); pair within runs
    order = np.lexsort((dI, rowA, g_all))
    gs, rs = g_all[order], rowA[order]
    newrun = np.ones(E, bool)
    newrun[1:] = (gs[1:] != gs[:-1]) | (rs[1:] != rs[:-1])
    run_id = np.cumsum(newrun) - 1
    run_start = np.flatnonzero(newrun)
    k_in_run = np.arange(E) - run_start[run_id]
    run_len = np.diff(np.concatenate([run_start, [E]]))
    is_single = (k_in_run == run_len[run_id] - 1) & (run_len[run_id] % 2 == 1)

    # paired edges: positions where k_in_run even and partner exists
    e0_pos = np.flatnonzero((k_in_run % 2 == 0) & ~is_single)
    e1_pos = e0_pos + 1
    pe0, pe1 = order[e0_pos], order[e1_pos]            # original edge ids
    pg = g_all[pe0]
    prow = rowA[pe0]
    dai = dI[pe0] + dI[pe1]                            # 0,1,2 == class of (da0,da1)
    pcls = 4 * dai + 2 * dJ[pe0] + dJ[pe1]             # [0,12)
    se = order[np.flatnonzero(is_single)]
    sg = g_all[se]
    scls = 2 * dI[se] + dJ[se]                         # [0,4)

    # ---- deal to cores per (g, bucket): pad each bucket globally to a
    # multiple of 8*32 (-1 sentinels) so every core gets an identical,
    # 32-aligned class layout.
    psort = np.lexsort((prow, pcls, pg))
    pgs, pcs = pg[psort], pcls[psort]
    pair_lists = [[None] * 4 for _ in range(NCORES)]   # (ids, cls) per (c, g)
    for g in range(4):
        ids_g = [[] for _ in range(NCORES)]
        cls_g = [[] for _ in range(NCORES)]
        for cl in range(12):
            sel = psort[(pgs == g) & (pcs == cl)]
            pad = (-len(sel)) % (NCORES * 32)
            if pad:
                sel = np.concatenate([sel, np.full(pad, -1, np.int64)])
            per = len(sel) // NCORES
            for c in range(NCORES):
                ids_g[c].append(sel[c * per:(c + 1) * per])
                cls_g[c].append(np.full(per, cl, np.int64))
        for c in range(NCORES):
            pair_lists[c][g] = (np.concatenate(ids_g[c]),
                                np.concatenate(cls_g[c]))
    ssort = np.lexsort((scls, sg))
    sgs, scs = sg[ssort], scls[ssort]
    sing_lists = [[None] * 4 for _ in range(NCORES)]
    for g in range(4):
        ids_g = [[] for _ in range(NCORES)]
        cls_g = [[] for _ in range(NCORES)]
        for cl in range(4):
            sel = ssort[(sgs == g) & (scs == cl)]
            pad = (-len(sel)) % (NCORES * 32)
            if pad:
                sel = np.concatenate([sel, np.full(pad, -1, np.int64)])
            per = len(sel) // NCORES
            for c in range(NCORES):
                ids_g[c].append(sel[c * per:(c + 1) * per])
                cls_g[c].append(np.full(per, cl, np.int64))
        for c in range(NCORES):
            sing_lists[c][g] = (np.concatenate(ids_g[c]),
                                np.concatenate(cls_g[c]))

    # ---- uniform plan (identical across cores by construction)
    def _tiles_for(cls_arr, cap):
        n = len(cls_arr)
        tiles = []
        for t0 in range(0, max(n, 1), cap):
            take = min(cap, n - t0)
            if take <= 0:
                break
            seg = cls_arr[t0:t0 + take]
            chg = np.flatnonzero(seg[1:] != seg[:-1]) + 1
            bnds = np.concatenate([[0], chg, [take]])
            segs = [(int(bnds[i]), int(bnds[i + 1]), int(seg[bnds[i]]))
                    for i in range(len(bnds) - 1)]
            tiles.append((take, _seg_runs(segs)))
        return tiles

    plan = []
    tile_take = []
    for g in range(4):
        for (take, runs) in _tiles_for(pair_lists[0][g][1], TP):
            plan.append((g, 0, runs))
            tile_take.append(take)
        for (take, runs) in _tiles_for(sing_lists[0][g][1], TS):
            plan.append((g, 1, runs))
            tile_take.append(take)
    plan = tuple(plan)
    nT = len(plan)
    totcols = sum(512 if t[1] == 0 else 256 for t in plan)

    if plan not in _PROGRAM_CACHE:
        _PROGRAM_CACHE.clear()
        _PROGRAM_CACHE[plan] = build_program(plan)
    nc = _PROGRAM_CACHE[plan]

    # ---- shared weight tensors (fp16)
    W1 = np.stack([W1s, W1d], axis=1).astype(F16)                  # (C, 2, HID)
    B1 = np.stack([b1s, b1d], axis=1).astype(np.float32)[:, :, None].reshape(HID, 2, 1)
    W2 = np.stack([W2s, W2d], axis=1).astype(F16)                  # (HID, 2, R)
    B2 = np.stack(
        [np.tile(b2s[None, :], (128, 1)), np.tile(b2d[None, :], (128, 1))], axis=1
    ).astype(np.float32)                                           # (128, 2, R)

    gbd = np.zeros((128, 96), np.float32)
    gT = np.asarray(gamma, np.float32).T                           # (R, L)
    for gblk in range(4):
        for b in range(B):
            gbd[32 * gblk + 16 * b:32 * gblk + 16 * b + 16,
                24 * gblk + 12 * b:24 * gblk + 12 * b + 12] = gT
    GBDh = gbd.astype(F16)

    # ---- per-core idx tables + unperm
    in_maps = []
    unperm = []
    for c in range(NCORES):
        wA = np.zeros((128, nT, 64), np.int16)
        wB = np.zeros((128, nT, 128), np.int16)
        eids = []
        rblocks = []
        cols = []
        col0 = 0
        pcur = [0] * 4
        scur = [0] * 4
        for Ti, (g, kind, runs) in enumerate(plan):
            nreal = tile_take[Ti]
            if kind == 0:
                ids, _ = pair_lists[c][g]
                take = ids[pcur[g]:pcur[g] + nreal]
                pcur[g] += nreal
                rowsA = np.zeros(1024, np.int64)
                flatB = np.zeros(2048, np.int64)
                m = np.arange(len(take))
                valid = take >= 0
                tv, mv = take[valid], m[valid]
                p0s, p1s = pe0[tv], pe1[tv]
                rowsA[mv] = prow[tv]
                p = mv % 128
                j = mv // 128
                q0 = p + 256 * j
                q1 = q0 + 128
                flatB[q0] = rowB[p0s]
                flatB[q1] = rowB[p1s]
                for (eid_arr, tslot) in ((p0s, 2 * j), (p1s, 2 * j + 1)):
                    eids.append(eid_arr)
                    rblocks.append(p // 32)
                    cols.append(col0 + 32 * tslot + (p % 32))
                wA[:, Ti, :] = _wrap_idx(rowsA.astype(np.int16))
                wB[:, Ti, :] = _wrap_idx(flatB.astype(np.int16))
                col0 += 512
            else:
                ids, _ = sing_lists[c][g]
                take = ids[scur[g]:scur[g] + nreal]
                scur[g] += nreal
                rowsA = np.zeros(1024, np.int64)
                flatB = np.zeros(1024, np.int64)
                m = np.arange(len(take))
                valid = take >= 0
                tv, mv = take[valid], m[valid]
                es = se[tv]
                rowsA[mv] = rowA[es]
                flatB[mv] = rowB[es]
                p = mv % 128
                j = mv // 128
                eids.append(es)
                rblocks.append(p // 32)
                cols.append(col0 + 32 * j + (p % 32))
                wA[:, Ti, :] = _wrap_idx(rowsA.astype(np.int16))
                wB[:, Ti, :64] = _wrap_idx(flatB.astype(np.int16))
                col0 += 256
        assert col0 == totcols
        eids = np.concatenate(eids) if eids else np.array([], np.int64)
        rblocks = np.concatenate(rblocks)
        cols = np.concatenate(cols)
        unperm.append((eids, rblocks, cols))

        HTs = np.ascontiguousarray(
            Hp[:, c * NP:(c + 1) * NP, :].transpose(2, 0, 1)
        ).astype(F16)                                              # (C, B, NP)
        in_maps.append({
            "HT": HTs, "W1": W1, "B1": B1, "W2": W2, "B2": B2,
            "GBD": GBDh, "IDXA": wA, "IDXB": wB,
        })

    import os
    import tempfile
    trace = bool(os.environ.get("BASS_KERNEL_TRACE"))
    tdir = None
    if trace:
        base = "/root/problem/work"
        tdir = tempfile.mkdtemp(prefix="ktrace_", dir=base if os.path.isdir(base) else None)
    res = run_bass_kernel_spmd(
        nc, in_maps, list(range(NCORES)), trace=trace, tmpdir=tdir,
    )
    if trace:
        kernel.last_trace_dir = tdir
        kernel.last_exec_time_ns = res.exec_time_ns

    logits = np.empty((B, L, E), np.float32)
    for c in range(NCORES):
        dev = np.asarray(res.results[c]["OUT"]).astype(np.float32)  # (96, totcols)
        dv = dev.reshape(4, 2, L, totcols)                          # (blk, b, l, col)
        eids, rblocks, cols = unperm[c]
        vals = dv[rblocks, :, :, cols]                              # (nv, 2, L)
        logits[:, :, eids] = vals.transpose(1, 2, 0)
    return logits
